# revision 29
# baseline (speedup 1.0000x reference)
"""BevPoolV2 Trainium2 kernel (8 NeuronCores, SPMD, no collectives) — v4.

Math: out[cell, :] = sum_{p: ranks_bev[p]=cell} depth_flat[ranks_depth[p]]
                     * feat_flat[ranks_feat[p], :]
with feat_flat = feat.transpose(0,1,3,4,2).reshape(-1, 128) and ranks_bev
sorted.  Output relaid to [b, c, z, h, w].

Distribution: core k owns cells [4096k, 4096(k+1)); each core writes a
disjoint [4096, 128] slab -- no collectives.

v4 design (replaces v0's scatter-add pipeline):
  - The whole per-core output [4096 cells, 128 C] f32 lives in PSUM
    (8 banks x [128, 512]): cell c -> partition c%128, bank (c//128)//4,
    col 128*((c//128)%4).  All accumulation is done by PE matmuls with
    start=False into memset-seeded banks; NO dma_scatter_add, NO chains.
  - Canonical static slot schedule (identical on all 8 cores, no
    data-dependent program): 1024 A slots = 8 slots x 128 windows of 32
    cells.  Slot t serves window w=t//8: its <=128 points (rf<32768,
    cell//32==w, in sorted order).  Per slot: gather feat rows (bf16,
    256B) + depth blocks (bf16, 256B, idx=rd//128), extract depth scalar
    with one DVE scalar_tensor_tensor (one-hot iota==rd%128, accum), form
    Ad = mask32 * d (bf16), matmul psum[32w%128 .. +32, block col] +=
    Ad^T @ G.
  - B slots (64 = 2 per 128-cell block) take the leftovers: points with
    rf>=32768 (int16 index limit) plus window-capacity overflow.  Their
    feat gather uses idx=rf//2 with elem=256 (two adjacent rows per
    descriptor, 512B); host-built even/odd one-hot masks [128, 2, 128]
    select the correct row parity via two matmuls per slot into the full
    128-cell block.
  - Readout: per-bank tensor_copy psum->SBUF + contiguous dma_start to
    DRAM out [128, 4096] (partition = cell%128, col = 128*(c//128) + C),
    interleaved right after the last chunk writing each bank.
  - Per-chunk aux data (packed int16 idx + bf16 masks + bf16 rd%128) is
    consolidated into ONE dma_start per chunk.  B compute runs before the
    A loop (its gathers are issued first on separate queues).

bf16 tables halve gather bytes; products accumulate in fp32 PSUM
(rel err ~2.4e-3, gate is 2e-2).

Host side: index/mask preprocessing (int arrays only -> packed int16
indices and one-hot masks), dtype/layout staging of the tables, final
relayout of core outputs.
"""
import numpy as np
import ml_dtypes

B, N, D, H, W = 2, 6, 120, 32, 88
C = 128
NCELLS = 32768
NCORES = 8
CELLS_PER_CORE = NCELLS // NCORES   # 4096
TILE_P = 128
A_LIM = 32768                       # feat rows handled by stream A (int16)
N_FEAT_ROWS = B * N * H * W         # 33792
N_DEPTH = B * N * D * H * W         # 4055040
N_DEP_BLK = N_DEPTH // 128          # 31680
N_FEAT2 = N_FEAT_ROWS // 2          # 16896 paired feat rows (stream B)

WIN = 32                            # cells per A window
NWIN = CELLS_PER_CORE // WIN        # 128 windows
SLOTS_PER_WIN = 8                   # A slots per window
NSLOT_A = NWIN * SLOTS_PER_WIN      # 1024
CHUNK = 32                          # A slots per chunk
NCHUNK = NSLOT_A // CHUNK           # 16
NBLK = CELLS_PER_CORE // 128        # 32 psum blocks
SLOTS_PER_BLK_B = 2
NSLOT_B = NBLK * SLOTS_PER_BLK_B    # 64
BBATCH = 32                         # B slots per sub-batch
NBBATCH = NSLOT_B // BBATCH         # 2

bf16 = ml_dtypes.bfloat16

# consolidated per-chunk aux buffer widths (int16 units):
# A: rfi(NIA/16) | rdi(NIA/16) | msk bf16 bits (CHUNK*WIN) | rdm bf16 (CHUNK)
AUXW_A = 2 * (CHUNK * TILE_P // 16) + CHUNK * WIN + CHUNK
# B: rfi | rdi | msk bf16 bits (BBATCH*256) | rdm bf16 (BBATCH)
AUXW_B = 2 * (BBATCH * TILE_P // 16) + BBATCH * 256 + BBATCH


def _pack16(ent):
    """entry i -> int16 storage [i%16, i//16], replicated to 128 partitions."""
    a = np.asarray(ent, np.int16).reshape(-1, 16).T
    return np.ascontiguousarray(np.tile(a, (8, 1)))


# ---------------------------------------------------------------- host prep
def _preprocess(ranks_depth, ranks_feat, ranks_bev):
    ranks_bev = np.asarray(ranks_bev)
    ranks_feat = np.asarray(ranks_feat).astype(np.int64)
    ranks_depth = np.asarray(ranks_depth).astype(np.int64)
    bounds = np.searchsorted(ranks_bev, np.arange(0, NCELLS + 1, CELLS_PER_CORE))
    cores = []
    for k in range(NCORES):
        lo, hi = int(bounds[k]), int(bounds[k + 1])
        rb = ranks_bev[lo:hi].astype(np.int64) - k * CELLS_PER_CORE
        rf = ranks_feat[lo:hi]
        rd = ranks_depth[lo:hi]
        isB = rf >= A_LIM

        # ---------------- stream A: fill canonical window slots ----------
        rbA, rfA, rdA = rb[~isB], rf[~isB], rd[~isB]
        wA = rbA // WIN
        rfA_s = np.zeros((NSLOT_A, TILE_P), np.int64)
        rdA_s = np.zeros((NSLOT_A, TILE_P), np.int64)
        rdmA_s = np.zeros((NSLOT_A, TILE_P), np.float32)
        mskA_s = np.zeros((NSLOT_A, TILE_P, WIN), np.float32)
        wstart = np.searchsorted(wA, np.arange(NWIN + 1))
        rank = np.arange(len(rbA)) - wstart[wA]   # position within window
        keep = rank < SLOTS_PER_WIN * TILE_P
        slot = wA[keep] * SLOTS_PER_WIN + rank[keep] // TILE_P
        p = rank[keep] % TILE_P
        rfA_s[slot, p] = rfA[keep]
        rdA_s[slot, p] = rdA[keep] // 128
        rdmA_s[slot, p] = rdA[keep] % 128
        mskA_s[slot, p, rbA[keep] - wA[keep] * WIN] = 1.0
        spill = ~keep

        # pack per chunk: one aux buffer [128, AUXW_A] int16 per chunk
        # layout: rfi | rdi | msk(bf16 bits) | rdm(bf16 bits)
        auxA = np.empty((NCHUNK, TILE_P, AUXW_A), np.int16)
        NI16 = CHUNK * TILE_P // 16
        for c in range(NCHUNK):
            s0 = c * CHUNK
            auxA[c, :, 0:NI16] = _pack16(rfA_s[s0:s0 + CHUNK].reshape(-1))
            auxA[c, :, NI16:2 * NI16] = _pack16(rdA_s[s0:s0 + CHUNK].reshape(-1))
            auxA[c, :, 2 * NI16:2 * NI16 + CHUNK * WIN] = mskA_s[
                s0:s0 + CHUNK].transpose(1, 0, 2).reshape(
                TILE_P, CHUNK * WIN).astype(bf16).view(np.int16)
            auxA[c, :, 2 * NI16 + CHUNK * WIN:] = rdmA_s[
                s0:s0 + CHUNK].T.astype(bf16).view(np.int16)

        # ---------------- stream B: leftovers per 128-cell block ---------
        rbB = np.concatenate([rb[isB], rbA[spill]])
        rfB = np.concatenate([rf[isB], rfA[spill]])
        rdB = np.concatenate([rd[isB], rdA[spill]])
        order = np.argsort(rbB, kind="stable")
        rbB, rfB, rdB = rbB[order], rfB[order], rdB[order]
        blkB = rbB // 128
        rfB_s = np.zeros((NSLOT_B, TILE_P), np.int64)   # idx = rf//2
        rdB_s = np.zeros((NSLOT_B, TILE_P), np.int64)
        rdmB_s = np.zeros((NSLOT_B, TILE_P), np.float32)
        mskB_s = np.zeros((NSLOT_B, TILE_P, 2, 128), np.float32)
        bstart = np.searchsorted(blkB, np.arange(NBLK + 1))
        rankB = np.arange(len(rbB)) - bstart[blkB]
        assert rankB.max(initial=0) < SLOTS_PER_BLK_B * TILE_P, (k,)
        slotB = blkB * SLOTS_PER_BLK_B + rankB // TILE_P
        pB = rankB % TILE_P
        rfB_s[slotB, pB] = rfB // 2
        rdB_s[slotB, pB] = rdB // 128
        rdmB_s[slotB, pB] = rdB % 128
        mskB_s[slotB, pB, rfB % 2, rbB - blkB * 128] = 1.0

        auxB = np.empty((NBBATCH, TILE_P, AUXW_B), np.int16)
        NB16 = BBATCH * TILE_P // 16
        for c in range(NBBATCH):
            s0 = c * BBATCH
            auxB[c, :, 0:NB16] = _pack16(rfB_s[s0:s0 + BBATCH].reshape(-1))
            auxB[c, :, NB16:2 * NB16] = _pack16(rdB_s[s0:s0 + BBATCH].reshape(-1))
            auxB[c, :, 2 * NB16:2 * NB16 + BBATCH * 256] = mskB_s[
                s0:s0 + BBATCH].transpose(1, 0, 2, 3).reshape(
                TILE_P, BBATCH * 256).astype(bf16).view(np.int16)
            auxB[c, :, 2 * NB16 + BBATCH * 256:] = rdmB_s[
                s0:s0 + BBATCH].T.astype(bf16).view(np.int16)

        cores.append(dict(auxA=auxA, auxB=auxB))
    return cores


# ---------------------------------------------------------------- program
_CACHED = {}


def _build_program(no_gather=False, no_stt=False, no_mm=False, no_ad=False):
    import concourse.bass as bass
    import concourse.bacc as bacc
    import concourse.tile as tile
    from concourse import mybir

    nc = bacc.Bacc("TRN2", target_bir_lowering=False, debug=False,
                   num_swdge_queues=4, dynamic_dma_scratch_size=49152)
    f32, i16, b16 = mybir.dt.float32, mybir.dt.int16, mybir.dt.bfloat16
    NIA = CHUNK * TILE_P                # 8192 idx per A chunk
    NIB = BBATCH * TILE_P               # 4096 idx per B batch
    feat_t = nc.dram_tensor("feat_tbl", [N_FEAT_ROWS, C], b16,
                            kind="ExternalInput").ap()
    feat2_t = nc.dram_tensor("feat2_tbl", [N_FEAT2, 2 * C], b16,
                             kind="ExternalInput").ap()
    dep_t = nc.dram_tensor("dep_tbl", [N_DEP_BLK, 128], b16,
                           kind="ExternalInput").ap()
    iota_t = nc.dram_tensor("iota", [TILE_P, 128], b16,
                            kind="ExternalInput").ap()
    auxA_t = nc.dram_tensor("auxA", [NCHUNK, TILE_P, AUXW_A], i16,
                            kind="ExternalInput").ap()
    auxB_t = nc.dram_tensor("auxB", [NBBATCH, TILE_P, AUXW_B], i16,
                            kind="ExternalInput").ap()
    out_t = nc.dram_tensor("out", [TILE_P, NBLK * C], f32,
                           kind="ExternalOutput").ap()

    EQ, MUL = mybir.AluOpType.is_equal, mybir.AluOpType.mult

    with tile.TileContext(nc) as tc:
        with (
            tc.tile_pool(name="cst", bufs=1) as cst,
            tc.tile_pool(name="ps", bufs=1, space="PSUM") as ps,
            tc.tile_pool(name="seq", bufs=2) as seq,
            tc.tile_pool(name="gp", bufs=2) as gp,
            tc.tile_pool(name="dp", bufs=2) as dp,
            tc.tile_pool(name="ex", bufs=4) as ex,
            tc.tile_pool(name="bb", bufs=1) as bb,
            tc.tile_pool(name="st", bufs=2) as st,
        ):
            iota_sb = cst.tile([TILE_P, 128], b16)
            nc.sync.dma_start(iota_sb[:], iota_t)

            banks = []
            for b in range(8):
                bank = ps.tile([TILE_P, 512], f32, tag=f"bank{b}",
                               name=f"bank{b}")
                banks.append(bank)
            for b in range(8):
                nc.vector.memset(banks[b][:], 0.0)

            def psum_slice(w):
                """32-cell window w -> psum [32, 128] slice."""
                blk = w // 4
                return banks[blk // 4][32 * (w % 4):32 * (w % 4) + 32,
                                       128 * (blk % 4):128 * (blk % 4) + 128]

            def blk_slice(blk):
                return banks[blk // 4][:, 128 * (blk % 4):128 * (blk % 4) + 128]

            # ---------------- stream B emitters (staged into the A loop) -
            NB16 = NIB // 16
            gB = {}

            def emit_b_gathers(c):
                aux_b = bb.tile([TILE_P, AUXW_B], i16, tag="auxB",
                                name=f"auxB{c}")
                nc.sync.dma_start(aux_b[:], auxB_t[c])
                rfi_b = aux_b[:, 0:NB16]
                rdi_b = aux_b[:, NB16:2 * NB16]
                msk_b = aux_b[:, 2 * NB16:2 * NB16 + BBATCH * 256].bitcast(b16)
                rdm_b = aux_b[:, 2 * NB16 + BBATCH * 256:].bitcast(b16)
                g_b = bb.tile([TILE_P, BBATCH * 2 * C], b16, tag="gB",
                              name=f"gB{c}")
                db_b = bb.tile([TILE_P, BBATCH * 128], b16, tag="dbB",
                               name=f"dbB{c}")
                nc.gpsimd.dma_gather(
                    g_b[:].rearrange("p (j e) -> p j e", e=2 * C),
                    feat2_t, rfi_b, NIB, NIB, 2 * C,
                    single_packet=False, queue_num=1)
                nc.gpsimd.dma_gather(
                    db_b[:].rearrange("p (j e) -> p j e", e=128),
                    dep_t, rdi_b, NIB, NIB, 128,
                    single_packet=False, queue_num=2)
                gB[c] = (rfi_b, rdi_b, msk_b, rdm_b, g_b, db_b)

            def emit_b_compute(c):
                rfi_b, rdi_b, msk_b, rdm_b, g_b, db_b = gB[c]
                gb3 = g_b[:].rearrange("p (j t e) -> p j t e", t=2, e=C)
                dbb3 = db_b[:].rearrange("p (j e) -> p j e", e=128)
                mb4 = msk_b.rearrange("p (j t e) -> p j t e", t=2, e=128)
                dB_sb = dp.tile([TILE_P, BBATCH], b16, tag="dB")
                for j in range(BBATCH):
                    scr = ex.tile([TILE_P, 128], b16, tag="scr")
                    nc.vector.scalar_tensor_tensor(
                        out=scr[:], in0=iota_sb[:],
                        scalar=rdm_b[:, j:j + 1],
                        in1=dbb3[:, j, :], op0=EQ, op1=MUL,
                        accum_out=dB_sb[:, j:j + 1])
                adB = dp.tile([TILE_P, BBATCH * 256], b16, tag="adB")
                adB4 = adB[:].rearrange("p (j t e) -> p j t e", t=2, e=128)
                nc.vector.tensor_tensor(
                    out=adB4, in0=mb4,
                    in1=dB_sb[:].to_broadcast([TILE_P, BBATCH, 2, 128]),
                    op=MUL)
                for j in range(BBATCH):
                    slot = c * BBATCH + j
                    blk = slot // SLOTS_PER_BLK_B
                    for par in range(2):
                        nc.tensor.matmul(out=blk_slice(blk),
                                         lhsT=adB4[:, j, par, :],
                                         rhs=gb3[:, j, par, :],
                                         start=False, stop=False,
                                         skip_group_check=True)

            # batch 0 (psum banks 0-3, read out from chunk 3 on) up front
            emit_b_gathers(0)
            emit_b_compute(0)

            # ---------------- stream A ----------------------------------
            NA16 = NIA // 16
            for c in range(NCHUNK):
                aux_sb = seq.tile([TILE_P, AUXW_A], i16, tag="aux")
                nc.sync.dma_start(aux_sb[:], auxA_t[c])
                rfi_sb = aux_sb[:, 0:NA16]
                rdi_sb = aux_sb[:, NA16:2 * NA16]
                msk_sb = aux_sb[:, 2 * NA16:2 * NA16 + CHUNK * WIN].bitcast(b16)
                rdm_sb = aux_sb[:, 2 * NA16 + CHUNK * WIN:].bitcast(b16)

                g_sb = gp.tile([TILE_P, CHUNK * C], b16, tag="g")
                db_sb = gp.tile([TILE_P, CHUNK * 128], b16, tag="db")
                g3 = g_sb[:].rearrange("p (j e) -> p j e", e=C)
                db3 = db_sb[:].rearrange("p (j e) -> p j e", e=128)
                if no_gather:
                    nc.vector.memset(g_sb[:, :1], 0.0)
                    nc.vector.memset(db_sb[:, :1], 0.0)
                else:
                    qf, qd = (0, 3) if c % 2 == 0 else (1, 2)
                    nc.gpsimd.dma_gather(g3, feat_t, rfi_sb, NIA, NIA, C,
                                         single_packet=False, queue_num=qf)
                    nc.gpsimd.dma_gather(db3, dep_t, rdi_sb, NIA, NIA, 128,
                                         single_packet=False, queue_num=qd)
                if c == 8:
                    emit_b_gathers(1)
                if c == 10:
                    emit_b_compute(1)

                d_sb = dp.tile([TILE_P, CHUNK], b16, tag="d")
                if no_stt:
                    nc.vector.memset(d_sb[:], 0.0)
                else:
                    for j in range(CHUNK):
                        scr = ex.tile([TILE_P, 128], b16, tag="scr")
                        nc.vector.scalar_tensor_tensor(
                            out=scr[:], in0=iota_sb[:],
                            scalar=rdm_sb[:, j:j + 1],
                            in1=db3[:, j, :], op0=EQ, op1=MUL,
                            accum_out=d_sb[:, j:j + 1])
                ad_sb = dp.tile([TILE_P, CHUNK * WIN], b16, tag="ad")
                ad3 = ad_sb[:].rearrange("p (j w) -> p j w", w=WIN)
                if no_ad:
                    nc.vector.memset(ad_sb[:, :1], 0.0)
                else:
                    nc.vector.tensor_tensor(
                        out=ad3,
                        in0=msk_sb.rearrange("p (j w) -> p j w", w=WIN),
                        in1=d_sb[:].to_broadcast([TILE_P, CHUNK, WIN]), op=MUL)

                if not no_mm:
                    for j in range(CHUNK):
                        slot = c * CHUNK + j
                        w = slot // SLOTS_PER_WIN
                        nc.tensor.matmul(out=psum_slice(w),
                                         lhsT=ad3[:, j, :],
                                         rhs=g3[:, j, :], start=False,
                                         stop=False, skip_group_check=True,
                                         tile_position=(0, 32 * (w % 4)))

                if not no_mm and c % 4 == 3:
                    b = c // 4
                    stage = st.tile([TILE_P, 512], f32, tag="stage")
                    nc.any.tensor_copy(stage[:], banks[b][:])
                    nc.sync.dma_start(out_t[:, 512 * b:512 * (b + 1)],
                                      stage[:])

            # readout is interleaved per-bank in the A loop (no_mm fallback)
            if no_mm:
                for b in range(8):
                    stage = st.tile([TILE_P, 512], f32, tag="stage")
                    nc.any.tensor_copy(stage[:], banks[b][:])
                    nc.sync.dma_start(out_t[:, 512 * b:512 * (b + 1)],
                                      stage[:])
    nc.compile()
    return nc


def _get_program():
    if "nc" not in _CACHED:
        _CACHED["nc"] = _build_program()
    return _CACHED["nc"]


# ---------------------------------------------------------------- entry
def kernel(depth, feat, ranks_depth, ranks_feat, ranks_bev,
           interval_starts=None, interval_lengths=None):
    import os
    from concourse import bass_utils

    depth = np.asarray(depth, dtype=np.float32)
    feat = np.asarray(feat, dtype=np.float32)
    feat_flat = np.ascontiguousarray(
        feat.transpose(0, 1, 3, 4, 2).reshape(-1, C).astype(bf16))
    feat2 = feat_flat.reshape(N_FEAT2, 2 * C)
    dep_blk = np.ascontiguousarray(depth.reshape(N_DEP_BLK, 128).astype(bf16))
    iota = np.ascontiguousarray(
        np.broadcast_to(np.arange(128, dtype=np.float32),
                        (TILE_P, 128)).astype(bf16))

    cores = _preprocess(ranks_depth, ranks_feat, ranks_bev)
    in_maps = []
    for k in range(NCORES):
        cd = cores[k]
        in_maps.append({
            "feat_tbl": feat_flat, "feat2_tbl": feat2, "dep_tbl": dep_blk,
            "iota": iota,
            "auxA": cd["auxA"], "auxB": cd["auxB"],
        })

    nc = _get_program()
    res = bass_utils.run_bass_kernel_spmd(nc, in_maps,
                                          core_ids=list(range(NCORES)),
                                          tmpdir=os.environ.get("BASS_TMPDIR"))
    _CACHED["last_results"] = res

    out_full = np.zeros((B, C, 1, 128, 128), np.float32)
    for k in range(NCORES):
        o = np.asarray(res.results[k]["out"])        # [128, 32*128]
        # cell c = 128*blk + p  ->  o[p, 128*blk + C]
        oc = o.reshape(TILE_P, NBLK, C).transpose(1, 0, 2).reshape(
            CELLS_PER_CORE, C)
        b, blk = k // 4, k % 4
        out_full[b, :, 0, 32 * blk:32 * (blk + 1), :] = \
            oc.T.reshape(C, 32, 128)
    return out_full


# revision 31
# speedup vs baseline: 1.1779x; 1.1779x over previous
"""BevPoolV2 Trainium2 kernel (8 NeuronCores, SPMD, no collectives) — v4.

Math: out[cell, :] = sum_{p: ranks_bev[p]=cell} depth_flat[ranks_depth[p]]
                     * feat_flat[ranks_feat[p], :]
with feat_flat = feat.transpose(0,1,3,4,2).reshape(-1, 128) and ranks_bev
sorted.  Output relaid to [b, c, z, h, w].

Distribution: core k owns cells [4096k, 4096(k+1)); each core writes a
disjoint [4096, 128] slab -- no collectives.

v4 design (replaces v0's scatter-add pipeline):
  - The whole per-core output [4096 cells, 128 C] f32 lives in PSUM
    (8 banks x [128, 512]): cell c -> partition c%128, bank (c//128)//4,
    col 128*((c//128)%4).  All accumulation is done by PE matmuls with
    start=False into memset-seeded banks; NO dma_scatter_add, NO chains.
  - Canonical static slot schedule (identical on all 8 cores, no
    data-dependent program): 1024 A slots = 8 slots x 128 windows of 32
    cells.  Slot t serves window w=t//8: its <=128 points (rf<32768,
    cell//32==w, in sorted order).  Per slot: gather feat rows (bf16,
    256B) + depth blocks (bf16, 256B, idx=rd//128), extract depth scalar
    with one DVE scalar_tensor_tensor (one-hot iota==rd%128, accum), form
    Ad = mask32 * d (bf16), matmul psum[32w%128 .. +32, block col] +=
    Ad^T @ G.
  - B slots (64 = 2 per 128-cell block) take the leftovers: points with
    rf>=32768 (int16 index limit) plus window-capacity overflow.  Their
    feat gather uses idx=rf//2 with elem=256 (two adjacent rows per
    descriptor, 512B); host-built even/odd one-hot masks [128, 2, 128]
    select the correct row parity via two matmuls per slot into the full
    128-cell block.
  - Readout: per-bank tensor_copy psum->SBUF + contiguous dma_start to
    DRAM out [128, 4096] (partition = cell%128, col = 128*(c//128) + C),
    interleaved right after the last chunk writing each bank.
  - Per-chunk aux data (packed int16 idx + bf16 masks + bf16 rd%128) is
    consolidated into ONE dma_start per chunk.  B compute runs before the
    A loop (its gathers are issued first on separate queues).

bf16 tables halve gather bytes; products accumulate in fp32 PSUM
(rel err ~2.4e-3, gate is 2e-2).

Host side: index/mask preprocessing (int arrays only -> packed int16
indices and one-hot masks), dtype/layout staging of the tables, final
relayout of core outputs.
"""
import numpy as np
import ml_dtypes

B, N, D, H, W = 2, 6, 120, 32, 88
C = 128
NCELLS = 32768
NCORES = 8
CELLS_PER_CORE = NCELLS // NCORES   # 4096
TILE_P = 128
A_LIM = 32768                       # feat rows handled by stream A (int16)
N_FEAT_ROWS = B * N * H * W         # 33792
N_DEPTH = B * N * D * H * W         # 4055040
N_DEP_BLK = N_DEPTH // 128          # 31680
N_FEAT2 = N_FEAT_ROWS // 2          # 16896 paired feat rows (stream B)

WIN = 32                            # cells per A window
NWIN = CELLS_PER_CORE // WIN        # 128 windows
SLOTS_PER_WIN = 8                   # A slots per window
NSLOT_A = NWIN * SLOTS_PER_WIN      # 1024
CHUNK = 32                          # A slots per chunk
NCHUNK = NSLOT_A // CHUNK           # 16
NBLK = CELLS_PER_CORE // 128        # 32 psum blocks
SLOTS_PER_BLK_B = 2
NSLOT_B = NBLK * SLOTS_PER_BLK_B    # 64
BBATCH = 32                         # B slots per sub-batch
NBBATCH = NSLOT_B // BBATCH         # 2

bf16 = ml_dtypes.bfloat16

# consolidated per-chunk aux buffer widths (int16 units):
# A: rfi(NIA/16) | rdi(NIA/16) | msk bf16 bits (CHUNK*WIN) | rdm bf16 (CHUNK)
AUXW_A = 2 * (CHUNK * TILE_P // 16) + CHUNK * WIN + CHUNK
# B: rfi | rdi | msk bf16 bits (BBATCH*256) | rdm bf16 (BBATCH)
AUXW_B = 2 * (BBATCH * TILE_P // 16) + BBATCH * 256 + BBATCH


def _pack16(ent):
    """entry i -> int16 storage [i%16, i//16], replicated to 128 partitions."""
    a = np.asarray(ent, np.int16).reshape(-1, 16).T
    return np.ascontiguousarray(np.tile(a, (8, 1)))


# ---------------------------------------------------------------- host prep
def _preprocess(ranks_depth, ranks_feat, ranks_bev):
    ranks_bev = np.asarray(ranks_bev)
    ranks_feat = np.asarray(ranks_feat).astype(np.int64)
    ranks_depth = np.asarray(ranks_depth).astype(np.int64)
    bounds = np.searchsorted(ranks_bev, np.arange(0, NCELLS + 1, CELLS_PER_CORE))
    cores = []
    for k in range(NCORES):
        lo, hi = int(bounds[k]), int(bounds[k + 1])
        rb = ranks_bev[lo:hi].astype(np.int64) - k * CELLS_PER_CORE
        rf = ranks_feat[lo:hi]
        rd = ranks_depth[lo:hi]
        isB = rf >= A_LIM

        # ---------------- stream A: fill canonical window slots ----------
        rbA, rfA, rdA = rb[~isB], rf[~isB], rd[~isB]
        wA = rbA // WIN
        rfA_s = np.zeros((NSLOT_A, TILE_P), np.int64)
        rdA_s = np.zeros((NSLOT_A, TILE_P), np.int64)
        rdmA_s = np.zeros((NSLOT_A, TILE_P), np.float32)
        mskA_s = np.zeros((NSLOT_A, TILE_P, WIN), np.float32)
        wstart = np.searchsorted(wA, np.arange(NWIN + 1))
        rank = np.arange(len(rbA)) - wstart[wA]   # position within window
        keep = rank < SLOTS_PER_WIN * TILE_P
        slot = wA[keep] * SLOTS_PER_WIN + rank[keep] // TILE_P
        p = rank[keep] % TILE_P
        rfA_s[slot, p] = rfA[keep]
        rdA_s[slot, p] = rdA[keep] // 128
        rdmA_s[slot, p] = rdA[keep] % 128
        mskA_s[slot, p, rbA[keep] - wA[keep] * WIN] = 1.0
        spill = ~keep

        # pack per chunk: one aux buffer [128, AUXW_A] int16 per chunk
        # layout: rfi | rdi | msk(bf16 bits) | rdm(bf16 bits)
        auxA = np.empty((NCHUNK, TILE_P, AUXW_A), np.int16)
        NI16 = CHUNK * TILE_P // 16
        for c in range(NCHUNK):
            s0 = c * CHUNK
            auxA[c, :, 0:NI16] = _pack16(rfA_s[s0:s0 + CHUNK].reshape(-1))
            auxA[c, :, NI16:2 * NI16] = _pack16(rdA_s[s0:s0 + CHUNK].reshape(-1))
            auxA[c, :, 2 * NI16:2 * NI16 + CHUNK * WIN] = mskA_s[
                s0:s0 + CHUNK].transpose(1, 0, 2).reshape(
                TILE_P, CHUNK * WIN).astype(bf16).view(np.int16)
            auxA[c, :, 2 * NI16 + CHUNK * WIN:] = rdmA_s[
                s0:s0 + CHUNK].T.astype(bf16).view(np.int16)

        # ---------------- stream B: leftovers per 128-cell block ---------
        rbB = np.concatenate([rb[isB], rbA[spill]])
        rfB = np.concatenate([rf[isB], rfA[spill]])
        rdB = np.concatenate([rd[isB], rdA[spill]])
        order = np.argsort(rbB, kind="stable")
        rbB, rfB, rdB = rbB[order], rfB[order], rdB[order]
        blkB = rbB // 128
        rfB_s = np.zeros((NSLOT_B, TILE_P), np.int64)   # idx = rf//2
        rdB_s = np.zeros((NSLOT_B, TILE_P), np.int64)
        rdmB_s = np.zeros((NSLOT_B, TILE_P), np.float32)
        mskB_s = np.zeros((NSLOT_B, TILE_P, 2, 128), np.float32)
        bstart = np.searchsorted(blkB, np.arange(NBLK + 1))
        rankB = np.arange(len(rbB)) - bstart[blkB]
        assert rankB.max(initial=0) < SLOTS_PER_BLK_B * TILE_P, (k,)
        slotB = blkB * SLOTS_PER_BLK_B + rankB // TILE_P
        pB = rankB % TILE_P
        rfB_s[slotB, pB] = rfB // 2
        rdB_s[slotB, pB] = rdB // 128
        rdmB_s[slotB, pB] = rdB % 128
        mskB_s[slotB, pB, rfB % 2, rbB - blkB * 128] = 1.0

        auxB = np.empty((NBBATCH, TILE_P, AUXW_B), np.int16)
        NB16 = BBATCH * TILE_P // 16
        for c in range(NBBATCH):
            s0 = c * BBATCH
            auxB[c, :, 0:NB16] = _pack16(rfB_s[s0:s0 + BBATCH].reshape(-1))
            auxB[c, :, NB16:2 * NB16] = _pack16(rdB_s[s0:s0 + BBATCH].reshape(-1))
            auxB[c, :, 2 * NB16:2 * NB16 + BBATCH * 256] = mskB_s[
                s0:s0 + BBATCH].transpose(1, 0, 2, 3).reshape(
                TILE_P, BBATCH * 256).astype(bf16).view(np.int16)
            auxB[c, :, 2 * NB16 + BBATCH * 256:] = rdmB_s[
                s0:s0 + BBATCH].T.astype(bf16).view(np.int16)

        cores.append(dict(auxA=auxA, auxB=auxB))
    return cores


# ---------------------------------------------------------------- program
_CACHED = {}


def _build_program(no_gather=False, no_stt=False, no_mm=False, no_ad=False):
    import concourse.bass as bass
    import concourse.bacc as bacc
    import concourse.tile as tile
    from concourse import mybir

    nc = bacc.Bacc("TRN2", target_bir_lowering=False, debug=False,
                   num_swdge_queues=4, dynamic_dma_scratch_size=32768)
    f32, i16, b16 = mybir.dt.float32, mybir.dt.int16, mybir.dt.bfloat16
    NIA = CHUNK * TILE_P                # 8192 idx per A chunk
    NIB = BBATCH * TILE_P               # 4096 idx per B batch
    feat_t = nc.dram_tensor("feat_tbl", [N_FEAT_ROWS, C], b16,
                            kind="ExternalInput").ap()
    feat2_t = nc.dram_tensor("feat2_tbl", [N_FEAT2, 2 * C], b16,
                             kind="ExternalInput").ap()
    dep_t = nc.dram_tensor("dep_tbl", [N_DEP_BLK, 128], b16,
                           kind="ExternalInput").ap()
    iota_t = nc.dram_tensor("iota", [TILE_P, 128], b16,
                            kind="ExternalInput").ap()
    auxA_t = nc.dram_tensor("auxA", [NCHUNK, TILE_P, AUXW_A], i16,
                            kind="ExternalInput").ap()
    auxB_t = nc.dram_tensor("auxB", [NBBATCH, TILE_P, AUXW_B], i16,
                            kind="ExternalInput").ap()
    out_t = nc.dram_tensor("out", [TILE_P, NBLK * C], f32,
                           kind="ExternalOutput").ap()

    EQ, MUL = mybir.AluOpType.is_equal, mybir.AluOpType.mult

    with tile.TileContext(nc) as tc:
        with (
            tc.tile_pool(name="cst", bufs=1) as cst,
            tc.tile_pool(name="ps", bufs=1, space="PSUM") as ps,
            tc.tile_pool(name="seq", bufs=2) as seq,
            tc.tile_pool(name="gp", bufs=2) as gp,
            tc.tile_pool(name="dp", bufs=2) as dp,
            tc.tile_pool(name="ex", bufs=4) as ex,
            tc.tile_pool(name="ab", bufs=1) as ab,
            tc.tile_pool(name="bb", bufs=1) as bb,
            tc.tile_pool(name="st", bufs=2) as st,
        ):
            iota_sb = cst.tile([TILE_P, 128], b16)
            nc.sync.dma_start(iota_sb[:], iota_t)

            banks = []
            for b in range(8):
                bank = ps.tile([TILE_P, 512], f32, tag=f"bank{b}",
                               name=f"bank{b}")
                banks.append(bank)
            for b in range(8):
                nc.vector.memset(banks[b][:], 0.0)

            def psum_slice(w):
                """32-cell window w -> psum [32, 128] slice."""
                blk = w // 4
                return banks[blk // 4][32 * (w % 4):32 * (w % 4) + 32,
                                       128 * (blk % 4):128 * (blk % 4) + 128]

            def blk_slice(blk):
                return banks[blk // 4][:, 128 * (blk % 4):128 * (blk % 4) + 128]

            # ---------------- stream B emitters (staged into the A loop) -
            NB16 = NIB // 16
            gB = {}

            def emit_b_gathers(c):
                aux_b = bb.tile([TILE_P, AUXW_B], i16, tag="auxB",
                                name=f"auxB{c}")
                nc.sync.dma_start(aux_b[:], auxB_t[c])
                rfi_b = aux_b[:, 0:NB16]
                rdi_b = aux_b[:, NB16:2 * NB16]
                msk_b = aux_b[:, 2 * NB16:2 * NB16 + BBATCH * 256].bitcast(b16)
                rdm_b = aux_b[:, 2 * NB16 + BBATCH * 256:].bitcast(b16)
                g_b = bb.tile([TILE_P, BBATCH * 2 * C], b16, tag="gB",
                              name=f"gB{c}")
                db_b = bb.tile([TILE_P, BBATCH * 128], b16, tag="dbB",
                               name=f"dbB{c}")
                nc.gpsimd.dma_gather(
                    g_b[:].rearrange("p (j e) -> p j e", e=2 * C),
                    feat2_t, rfi_b, NIB, NIB, 2 * C,
                    single_packet=False, queue_num=1)
                nc.gpsimd.dma_gather(
                    db_b[:].rearrange("p (j e) -> p j e", e=128),
                    dep_t, rdi_b, NIB, NIB, 128,
                    single_packet=False, queue_num=2)
                gB[c] = (rfi_b, rdi_b, msk_b, rdm_b, g_b, db_b)

            def emit_b_compute(c):
                rfi_b, rdi_b, msk_b, rdm_b, g_b, db_b = gB[c]
                gb3 = g_b[:].rearrange("p (j t e) -> p j t e", t=2, e=C)
                dbb3 = db_b[:].rearrange("p (j e) -> p j e", e=128)
                mb4 = msk_b.rearrange("p (j t e) -> p j t e", t=2, e=128)
                dB_sb = dp.tile([TILE_P, BBATCH], b16, tag="dB")
                for j in range(BBATCH):
                    scr = ex.tile([TILE_P, 128], b16, tag="scr")
                    nc.vector.scalar_tensor_tensor(
                        out=scr[:], in0=iota_sb[:],
                        scalar=rdm_b[:, j:j + 1],
                        in1=dbb3[:, j, :], op0=EQ, op1=MUL,
                        accum_out=dB_sb[:, j:j + 1])
                adB = ab.tile([TILE_P, BBATCH * 256], b16, tag="adB")
                adB4 = adB[:].rearrange("p (j t e) -> p j t e", t=2, e=128)
                nc.vector.tensor_tensor(
                    out=adB4, in0=mb4,
                    in1=dB_sb[:].to_broadcast([TILE_P, BBATCH, 2, 128]),
                    op=MUL)
                for j in range(BBATCH):
                    slot = c * BBATCH + j
                    blk = slot // SLOTS_PER_BLK_B
                    for par in range(2):
                        nc.tensor.matmul(out=blk_slice(blk),
                                         lhsT=adB4[:, j, par, :],
                                         rhs=gb3[:, j, par, :],
                                         start=False, stop=False,
                                         skip_group_check=True)

            # batch 0 (psum banks 0-3, read out from chunk 3 on) up front
            emit_b_gathers(0)
            emit_b_compute(0)

            # ---------------- stream A ----------------------------------
            NA16 = NIA // 16
            for c in range(NCHUNK):
                aux_sb = seq.tile([TILE_P, AUXW_A], i16, tag="aux")
                nc.sync.dma_start(aux_sb[:], auxA_t[c])
                rfi_sb = aux_sb[:, 0:NA16]
                rdi_sb = aux_sb[:, NA16:2 * NA16]
                msk_sb = aux_sb[:, 2 * NA16:2 * NA16 + CHUNK * WIN].bitcast(b16)
                rdm_sb = aux_sb[:, 2 * NA16 + CHUNK * WIN:].bitcast(b16)

                g_sb = gp.tile([TILE_P, CHUNK * C], b16, tag="g")
                db_sb = gp.tile([TILE_P, CHUNK * 128], b16, tag="db")
                g3 = g_sb[:].rearrange("p (j e) -> p j e", e=C)
                db3 = db_sb[:].rearrange("p (j e) -> p j e", e=128)
                if no_gather:
                    nc.vector.memset(g_sb[:, :1], 0.0)
                    nc.vector.memset(db_sb[:, :1], 0.0)
                else:
                    qf, qd = (0, 3) if c % 2 == 0 else (1, 2)
                    nc.gpsimd.dma_gather(g3, feat_t, rfi_sb, NIA, NIA, C,
                                         single_packet=False, queue_num=qf)
                    nc.gpsimd.dma_gather(db3, dep_t, rdi_sb, NIA, NIA, 128,
                                         single_packet=False, queue_num=qd)
                if c == 8:
                    emit_b_gathers(1)
                if c == 10:
                    emit_b_compute(1)

                d_sb = dp.tile([TILE_P, CHUNK], b16, tag="d")
                if no_stt:
                    nc.vector.memset(d_sb[:], 0.0)
                else:
                    for j in range(CHUNK):
                        scr = ex.tile([TILE_P, 128], b16, tag="scr")
                        nc.vector.scalar_tensor_tensor(
                            out=scr[:], in0=iota_sb[:],
                            scalar=rdm_sb[:, j:j + 1],
                            in1=db3[:, j, :], op0=EQ, op1=MUL,
                            accum_out=d_sb[:, j:j + 1])
                ad_sb = dp.tile([TILE_P, CHUNK * WIN], b16, tag="ad")
                ad3 = ad_sb[:].rearrange("p (j w) -> p j w", w=WIN)
                if no_ad:
                    nc.vector.memset(ad_sb[:, :1], 0.0)
                else:
                    nc.vector.tensor_tensor(
                        out=ad3,
                        in0=msk_sb.rearrange("p (j w) -> p j w", w=WIN),
                        in1=d_sb[:].to_broadcast([TILE_P, CHUNK, WIN]), op=MUL)

                if not no_mm:
                    for j in range(CHUNK):
                        slot = c * CHUNK + j
                        w = slot // SLOTS_PER_WIN
                        nc.tensor.matmul(out=psum_slice(w),
                                         lhsT=ad3[:, j, :],
                                         rhs=g3[:, j, :], start=False,
                                         stop=False, skip_group_check=True,
                                         tile_position=(0, 32 * (w % 4)))

                if not no_mm and c % 4 == 3:
                    b = c // 4
                    stage = st.tile([TILE_P, 512], f32, tag="stage")
                    nc.any.tensor_copy(stage[:], banks[b][:])
                    nc.sync.dma_start(out_t[:, 512 * b:512 * (b + 1)],
                                      stage[:])

            # readout is interleaved per-bank in the A loop (no_mm fallback)
            if no_mm:
                for b in range(8):
                    stage = st.tile([TILE_P, 512], f32, tag="stage")
                    nc.any.tensor_copy(stage[:], banks[b][:])
                    nc.sync.dma_start(out_t[:, 512 * b:512 * (b + 1)],
                                      stage[:])
    nc.compile()
    return nc


def _get_program():
    if "nc" not in _CACHED:
        _CACHED["nc"] = _build_program()
    return _CACHED["nc"]


# ---------------------------------------------------------------- entry
def kernel(depth, feat, ranks_depth, ranks_feat, ranks_bev,
           interval_starts=None, interval_lengths=None):
    import os
    from concourse import bass_utils

    depth = np.asarray(depth, dtype=np.float32)
    feat = np.asarray(feat, dtype=np.float32)
    feat_flat = np.ascontiguousarray(
        feat.transpose(0, 1, 3, 4, 2).reshape(-1, C).astype(bf16))
    feat2 = feat_flat.reshape(N_FEAT2, 2 * C)
    dep_blk = np.ascontiguousarray(depth.reshape(N_DEP_BLK, 128).astype(bf16))
    iota = np.ascontiguousarray(
        np.broadcast_to(np.arange(128, dtype=np.float32),
                        (TILE_P, 128)).astype(bf16))

    cores = _preprocess(ranks_depth, ranks_feat, ranks_bev)
    in_maps = []
    for k in range(NCORES):
        cd = cores[k]
        in_maps.append({
            "feat_tbl": feat_flat, "feat2_tbl": feat2, "dep_tbl": dep_blk,
            "iota": iota,
            "auxA": cd["auxA"], "auxB": cd["auxB"],
        })

    nc = _get_program()
    res = bass_utils.run_bass_kernel_spmd(nc, in_maps,
                                          core_ids=list(range(NCORES)),
                                          tmpdir=os.environ.get("BASS_TMPDIR"))
    _CACHED["last_results"] = res

    out_full = np.zeros((B, C, 1, 128, 128), np.float32)
    for k in range(NCORES):
        o = np.asarray(res.results[k]["out"])        # [128, 32*128]
        # cell c = 128*blk + p  ->  o[p, 128*blk + C]
        oc = o.reshape(TILE_P, NBLK, C).transpose(1, 0, 2).reshape(
            CELLS_PER_CORE, C)
        b, blk = k // 4, k % 4
        out_full[b, :, 0, 32 * blk:32 * (blk + 1), :] = \
            oc.T.reshape(C, 32, 128)
    return out_full
